# revision 15
# baseline (speedup 1.0000x reference)
"""Bilateral slice apply (HDRNet, has_offset=True) on 8 TRN2 NeuronCores.

Contract: kernel(**inputs) takes FULL inputs, returns FULL output.
  bilateral_grid [4,16,16,8,12] f32, guide [4,1024,1024] f32,
  input [4,1024,1024,3] f32 -> out [4,1024,1024,3] f32.

Strategy ("XTAB v7"): shard H across the 8 cores (128 rows x 4 batches
each). The trilinear slice is decomposed per pixel as
  coeffs_k(p) = C(y,x,k) + sum_{d=0..6} Delta_d(y,x,k) * r_d(u)
  r_d(u) = clip(u, d+.5, d+1.5) - (d+1),  u = 8*guide
The (C, Delta) tables absorb the (x, y) bilinear interpolation. The
host reduces the tiny grid to per-row piecewise-linear-in-x coarse
tables  T(row, x) = A(row, xb) + xi(x)*B(row, xb)  (y-interp folded on
host, 17 x-nodes). The device expands them to full x resolution with
two-scalar tensor_scalar ops (A,B ride the per-partition scalar slots;
work split across VectorE / GpSimd / ScalarE), evaluates the
z-piecewise-linear basis per pixel in fp16 on VectorE, and applies the
per-pixel affine transform on GpSimd, fp16 with an fp32 finish.
"""

import os
import numpy as np

_NCORES = 8
B, H, W, CIN = 4, 1024, 1024, 3
GH, GW, GD, GC = 16, 16, 8, 12
ROWS = H // _NCORES          # rows per core per batch = 128
NZ = 8                       # z-basis slots: [C, Delta_0..Delta_6]
NK = GC                      # 12 coefficient channels
NDK = NZ * NK                # 96
NXB = 17                     # x nodes (piecewise-linear knots)
XH = 2                       # x halves
XW = W // XH                 # 512
KCH = 6                      # dk chunks per x half (2 k each)
KPC = NK // KCH              # 2 k per chunk
DKC = NZ * KPC               # 16 dk per chunk
# x pieces per half: (node, x0 relative to half, width)
XPIECES = [
    [(0, 0, 32)] + [(n, 64 * n - 32, 64) for n in range(1, 8)] + [(8, 480, 32)],
    [(8, 0, 32)] + [(n, 64 * n - 32 - 512, 64) for n in range(9, 16)]
    + [(16, 480, 32)],
]

_cache = {}


def _host_tables(grid):
    """grid [B,GH,GW,GD,GC] f32 ->
    tabs [B, H, 2, NDK, NXB] f32 (per-row folded coarse tables)
    + xi_rep [ROWS, W] fp16 (within-block x fraction, same every core)."""
    g = grid.astype(np.float64)
    Delta = g[..., 1:, :] - g[..., :-1, :]                    # [B,GH,GW,7,GC]
    C = g[..., 0, :] + 0.5 * Delta.sum(axis=-2)               # [B,GH,GW,GC]
    Tz = np.concatenate([C[..., None, :], Delta], axis=-2)    # [B,GH,GW,NZ,GC]
    # dk = k*NZ + z  -> [B, GH, GW(jx), NDK]
    Tz = np.transpose(Tz, (0, 1, 2, 4, 3)).reshape(B, GH, GW, NDK)

    # coarse x nodes: block n covers fx = n-1 -> taps jx0=clip(n-1), jx1=clip(n)
    n = np.arange(NXB)
    jx0 = np.clip(n - 1, 0, GW - 1)
    jx1 = np.clip(n, 0, GW - 1)
    A = Tz[:, :, jx0]                                         # [B,GH,NXB,NDK]
    Bt = Tz[:, :, jx1] - Tz[:, :, jx0]

    # per-row y fold
    y = np.arange(H)
    gy = (y + 0.5) * (GH / H)
    fy = np.floor(gy - 0.5)
    eta = gy - 0.5 - fy
    jy0 = np.clip(fy, 0, GH - 1).astype(int)
    jy1 = np.clip(fy + 1, 0, GH - 1).astype(int)
    # [B, H, NXB, NDK]
    Arow = A[:, jy0] + eta[None, :, None, None] * (A[:, jy1] - A[:, jy0])
    Brow = Bt[:, jy0] + eta[None, :, None, None] * (Bt[:, jy1] - Bt[:, jy0])
    # -> [B, H, 2, NDK, NXB] f32
    tabs = np.stack([Arow, Brow], axis=2).transpose(0, 1, 2, 4, 3)
    tabs = np.ascontiguousarray(tabs, dtype=np.float32)

    x = np.arange(W)
    gx = (x + 0.5) * (GW / W)
    fx = np.floor(gx - 0.5)
    xi = (gx - 0.5 - fx).astype(np.float16)                   # [W]
    xi_rep = np.broadcast_to(xi, (ROWS, W)).copy()
    return tabs, xi_rep


def _build_bass():
    from contextlib import ExitStack
    import concourse.bacc as bacc
    import concourse.bass as bass
    import concourse.tile as tile
    import concourse.mybir as mybir

    f32 = mybir.dt.float32
    f16 = mybir.dt.float16
    Alu = mybir.AluOpType
    Act = mybir.ActivationFunctionType

    nc = bacc.Bacc("TRN2", debug=False)
    tabs = nc.dram_tensor("tabs", [B, ROWS, 2, NDK, NXB], f32,
                          kind="ExternalInput").ap()
    guide = nc.dram_tensor("guide_s", [B, ROWS, W], f32, kind="ExternalInput").ap()
    inp = nc.dram_tensor("input_s", [B, CIN, ROWS, W], f32, kind="ExternalInput").ap()
    xi_d = nc.dram_tensor("xi_rep", [ROWS, W], f16, kind="ExternalInput").ap()
    out = nc.dram_tensor("out_s", [B, CIN, ROWS, W], f32, kind="ExternalOutput").ap()

    with ExitStack() as ctx:
        tc = ctx.enter_context(tile.TileContext(nc))
        singles = ctx.enter_context(tc.tile_pool(name="singles", bufs=1))
        gpool = ctx.enter_context(tc.tile_pool(name="gpool", bufs=2))
        uvpool = ctx.enter_context(tc.tile_pool(name="uvpool", bufs=1))
        cpool = ctx.enter_context(tc.tile_pool(name="cpool", bufs=1))
        rpool = ctx.enter_context(tc.tile_pool(name="rpool", bufs=1))
        kpool = ctx.enter_context(tc.tile_pool(name="kpool", bufs=2))
        tpool = ctx.enter_context(tc.tile_pool(name="tpool", bufs=2))
        mpool = ctx.enter_context(tc.tile_pool(name="mpool", bufs=2))
        apool = ctx.enter_context(tc.tile_pool(name="apool", bufs=2))
        opool = ctx.enter_context(tc.tile_pool(name="opool", bufs=2))

        xi_t = singles.tile([ROWS, W], f16)
        nc.sync.dma_start(out=xi_t, in_=xi_d)

        for b in range(B):
            g_t = gpool.tile([ROWS, W], f32, tag="g")
            nc.scalar.dma_start(out=g_t, in_=guide[b])
            u_t = uvpool.tile([ROWS, W], f32, tag="u")
            nc.vector.tensor_scalar_mul(u_t, g_t, float(GD))

            # coarse per-row tables for this batch: [ROWS, 2, NDK, NXB] f32
            kt = kpool.tile([ROWS, 2, NDK, NXB], f32, tag="kt")
            nc.sync.dma_start(out=kt, in_=tabs[b])

            rs = []
            for d in range(7):
                v_t = uvpool.tile([ROWS, W], f32, tag="v")
                nc.vector.tensor_scalar(
                    v_t, u_t, d + 0.5, d + 1.5, Alu.max, Alu.min
                )
                r_t = rpool.tile([ROWS, W], f16, tag=f"r{d}")
                nc.vector.tensor_scalar_sub(r_t, v_t, float(d + 1))
                rs.append(r_t)

            c_ts = []
            for ch in range(CIN):
                c_t = cpool.tile([ROWS, W], f32, tag=f"c{ch}")
                nc.scalar.dma_start(out=c_t, in_=inp[b, ch])
                c16 = cpool.tile([ROWS, W], f16, tag=f"c16_{ch}")
                nc.scalar.copy(c16, c_t)
                c_ts.append(c16)

            for xh in range(XH):
                xsl = slice(XW * xh, XW * (xh + 1))
                acc = apool.tile([ROWS, NK, XW], f16, tag="acc")
                for chunk in range(KCH):
                    dk0 = DKC * chunk
                    tf = tpool.tile([ROWS, DKC, XW], f16, tag="tf")
                    # x expansion: tf[:, j, x] = xi(x)*B[dk0+j, node] + A[...]
                    # split across engines by dk row
                    for j in range(DKC):
                        dk = dk0 + j
                        for (node, x0, wid) in XPIECES[xh]:
                            a_s = kt[:, 0, dk, node:node + 1]
                            b_s = kt[:, 1, dk, node:node + 1]
                            xi_sl = xi_t[:, XW * xh + x0: XW * xh + x0 + wid]
                            dst = tf[:, j, x0:x0 + wid]
                            if j >= 13:
                                nc.scalar.activation(dst, xi_sl, Act.Identity,
                                                     bias=a_s, scale=b_s)
                            elif j >= 9:
                                nc.gpsimd.tensor_scalar(
                                    dst, xi_sl, b_s, a_s, Alu.mult, Alu.add)
                            else:
                                nc.vector.tensor_scalar(
                                    dst, xi_sl, b_s, a_s, Alu.mult, Alu.add)
                    # z-eval for the KPC k's of this chunk (batched over k)
                    ps = []
                    for d in range(7):
                        rsl_ap = rs[d][:, xsl]
                        rb = bass.AP(
                            tensor=rsl_ap.tensor, offset=rsl_ap.offset,
                            ap=[list(rsl_ap.ap[0]), [0, KPC]]
                            + [list(rsl_ap.ap[1])],
                        )
                        p_t = mpool.tile([ROWS, KPC, XW], f16, tag=f"p{d}")
                        nc.vector.tensor_mul(p_t, rb, tf[:, 1 + d:DKC:NZ])
                        ps.append(p_t)
                    t0 = mpool.tile([ROWS, KPC, XW], f16, tag="t0")
                    t1 = mpool.tile([ROWS, KPC, XW], f16, tag="t1")
                    t2 = mpool.tile([ROWS, KPC, XW], f16, tag="t2")
                    t3 = mpool.tile([ROWS, KPC, XW], f16, tag="t3")
                    nc.vector.tensor_add(t0, ps[0], ps[1])
                    nc.vector.tensor_add(t1, ps[2], ps[3])
                    nc.vector.tensor_add(t2, ps[4], ps[5])
                    nc.vector.tensor_add(t3, ps[6], tf[:, 0:DKC:NZ])
                    nc.vector.tensor_add(t0, t0, t1)
                    nc.vector.tensor_add(t2, t2, t3)
                    nc.vector.tensor_add(acc[:, KPC * chunk:KPC * (chunk + 1)],
                                         t0, t2)
                # apply on GpSimd: out_o = sum_c A[o*4+c]*inp_c + A[o*4+3]
                for o in range(CIN):
                    m0 = mpool.tile([ROWS, XW], f16, tag="m0")
                    m1 = mpool.tile([ROWS, XW], f16, tag="m1")
                    m2 = mpool.tile([ROWS, XW], f16, tag="m2")
                    oo = opool.tile([ROWS, XW], f32, tag=f"oo{o}", name=f"oo{o}")
                    nc.gpsimd.tensor_mul(m0, acc[:, 4 * o + 0], c_ts[0][:, xsl])
                    nc.gpsimd.tensor_mul(m1, acc[:, 4 * o + 1], c_ts[1][:, xsl])
                    nc.gpsimd.tensor_mul(m2, acc[:, 4 * o + 2], c_ts[2][:, xsl])
                    nc.gpsimd.tensor_add(m0, m0, m1)
                    nc.gpsimd.tensor_add(m2, m2, acc[:, 4 * o + 3])
                    nc.gpsimd.tensor_add(oo, m0, m2)
                    nc.scalar.dma_start(out=out[b, o, :, xsl], in_=oo)

    nc.compile()
    return nc


def kernel(bilateral_grid, guide, input):
    from concourse.bass_utils import run_bass_kernel_spmd

    grid = np.asarray(bilateral_grid, np.float32)
    guide = np.asarray(guide, np.float32)
    inp = np.asarray(input, np.float32)

    tabs_full, xi_rep = _host_tables(grid)                    # [B,H,2,NDK,NXB]
    inp_pl = np.ascontiguousarray(np.moveaxis(inp, 3, 1))     # [B,CIN,H,W]

    in_maps = []
    for core in range(_NCORES):
        rsl = slice(ROWS * core, ROWS * (core + 1))
        in_maps.append({
            "tabs": np.ascontiguousarray(tabs_full[:, rsl]),
            "guide_s": np.ascontiguousarray(guide[:, rsl]),
            "input_s": np.ascontiguousarray(inp_pl[:, :, rsl]),
            "xi_rep": xi_rep,
        })

    if "nc" not in _cache:
        _cache["nc"] = _build_bass()
    nc = _cache["nc"]

    trace = bool(int(os.environ.get("BILATERAL_TRACE", "0")))
    if trace:
        import sys, types
        sys.path.insert(0, "/root/.axon_site")
        try:
            from trn_agent_boot.trn_boot import _ntff_profile_via_ctypes
            m = types.ModuleType("antenv.axon_hooks")
            m.get_axon_ntff_profile_hook = (
                lambda: _ntff_profile_via_ctypes("/opt/axon/libaxon_pjrt.so")
            )
            sys.modules["antenv.axon_hooks"] = m
        except Exception:
            trace = False

    res = run_bass_kernel_spmd(nc, in_maps, list(range(_NCORES)), trace=trace)
    _cache["last_res"] = res
    if trace and res.exec_time_ns is not None:
        print(f"HW exec time: {res.exec_time_ns} ns "
              f"(mean {res.mean_exec_time_ns} ns)")

    out = np.empty((B, H, W, CIN), np.float32)
    for core in range(_NCORES):
        rsl = slice(ROWS * core, ROWS * (core + 1))
        out[:, rsl] = np.moveaxis(res.results[core]["out_s"], 1, 3)
    return out
